# revision 7
# baseline (speedup 1.0000x reference)
"""Trainium2 Bass kernel for nn_MixtureCogrammar.

Computation (reference):
    attn  = softmax(morphosyn @ W_affix)                    [B, V]
    affix = attn @ affix_vocab.reshape(V, D*N)              [B, D, N]
    wC    = cumsum_n( sum_{ijk} a_i b_j f_k softmax(pivot_logits[i,j,:,k,:]) )
    out   = stem + wC * (affix - stem)

Distribution: D is sharded over the 8 cores (D_local = 32). Every core
computes the full attention (cheap); the pivot/wC path is batch-sharded
with an AllGather; affix_vocab / stem / out are D-sharded.

Per-core structure (v2):
  - pivot softmax -> wC fires the AllGather as early as possible (it is
    the only cross-core dependency and pays the launch-skew barrier)
  - attention: logits come out of the PE already log-softmax-normalized
    (a K=8 row-selector matmul subtracts ln(sum(exp)) inside the psum
    accumulation), so exp() on ScalarE writes *normalized* fp8 attnT
    directly -- no transposes of attn, no per-partition rescale later
  - the big matmul runs fp8 DoubleRow (contraction 256/MM, 2 MMs per
    512-col psum group instead of 4 bf16 MMs), and a third bf16 matmul
    with -I weights subtracts stem inside the accumulation, so PSUM
    holds delta = affix - stem directly
  - ScalarE evacuates delta (psum->bf16 copy); DVE only does
    prod = delta*wC and out = prod + stem; output streams out per
    2048-col tile
"""

import os
import sys

import numpy as np

for _p in ("/opt/trn_rl_repo",):
    if os.path.isdir(_p) and _p not in sys.path:
        sys.path.append(_p)

import concourse.bass as bass  # noqa: E402
import concourse.tile as tile  # noqa: E402
from concourse import bacc, mybir  # noqa: E402
from concourse.bass import ts  # noqa: E402
from concourse.bass_utils import run_bass_kernel_spmd  # noqa: E402
from concourse.masks import make_identity  # noqa: E402

import ml_dtypes  # noqa: E402

B, D, N, DM, V = 1024, 256, 256, 128, 512
NCORES = 8
DLOC = D // NCORES          # 32 d-values per core
BCH = B // 128              # 8 batch chunks
DN = DLOC * N               # 8192 free elems per core
HALF = DN // 2              # 4096 per round
DHALF = DLOC // 2           # 16 d-values per round
PSW = 2048                  # one psum tile = 4 banks

F32 = mybir.dt.float32
F32R = mybir.dt.float32r
BF16 = mybir.dt.bfloat16
FP8 = mybir.dt.float8e4
EXP = mybir.ActivationFunctionType.Exp
LN = mybir.ActivationFunctionType.Ln
COPY = mybir.ActivationFunctionType.Copy
ALU = mybir.AluOpType
DR = mybir.MatmulPerfMode.DoubleRow

# knobs
USE_DR = True        # fp8 DoubleRow for the big matmul (else bf16)
STEM_PE_MOD = 1      # 1: every tile subtracts stem via -I matmul;
                     # k>1: only tiles with gi%k==0; 0: never (DVE sub)
PIVOT_BF16 = True    # host-cast pivot logits to bf16

LAST_RESULT = None   # BassKernelResults of the last run (exec_time_ns etc.)

_CACHE = {}


def _build():
    key = (USE_DR, STEM_PE_MOD, PIVOT_BF16)
    if key in _CACHE:
        return _CACHE[key]

    vocab_dt = FP8 if USE_DR else BF16
    pivot_dt = BF16 if PIVOT_BF16 else F32

    nc = bacc.Bacc("TRN2", target_bir_lowering=False, debug=False,
                   num_devices=NCORES)

    stem_d = nc.dram_tensor("stem", [B, DLOC, N], BF16, kind="ExternalInput").ap()
    vocab_d = nc.dram_tensor("vocab", [V, DLOC, N], vocab_dt, kind="ExternalInput").ap()
    mor_d = nc.dram_tensor("morpho", [B, DM], F32R, kind="ExternalInput").ap()
    waff_d = nc.dram_tensor("waffix", [DM, V], F32R, kind="ExternalInput").ap()
    pv_d = nc.dram_tensor("pivot", [2, 2, 128, 5, N], pivot_dt, kind="ExternalInput").ap()
    abf_d = nc.dram_tensor("abf", [1, 9], F32, kind="ExternalInput").ap()
    nsel_d = nc.dram_tensor("negsel", [8, 8, 128], F32R, kind="ExternalInput").ap()
    out_d = nc.dram_tensor("out", [B, DLOC, N], BF16, kind="ExternalOutput").ap()

    from contextlib import ExitStack

    with tile.TileContext(nc) as tc, ExitStack() as ctx:
        const = ctx.enter_context(tc.tile_pool(name="const", bufs=1))

        ident = const.tile([128, 128], F32)
        make_identity(nc, ident[:, :])
        negI = const.tile([128, 128], BF16)
        nc.vector.tensor_scalar_mul(negI[:, :], ident[:, :], -1.0)

        attnT = const.tile([128, 4, B], FP8)       # [v_part, vc, b] normalized
        wc_sb = const.tile([128, BCH, N], BF16)    # [b_part, cb, n]
        w_bcast = const.tile([128, 20], F32)
        wsb = const.tile([128, V], F32R)           # W_affix resident
        morT_all = const.tile([128, BCH, 128], F32R)  # [dm, cb, b] via DMA transpose
        sE_all = const.tile([128, BCH], F32)       # sum(exp(logits)) per b
        lnS_all = const.tile([8, 128], F32R)       # ln of the above, [cb, b]
        negsel = const.tile([8, 8, 128], F32R)     # -row-selector weights

        # ---------- input DMAs ----------
        small = ctx.enter_context(tc.tile_pool(name="small", bufs=1))
        bp = ctx.enter_context(tc.tile_pool(name="attn", bufs=2))
        pvp = tc.alloc_tile_pool(name="pv", bufs=1)
        pv = pvp.tile([128, 4, 5, N], pivot_dt)
        abf = small.tile([1, 9], F32)
        nc.sync.dma_start(abf[0:1, :], abf_d[:, :])
        for ij in range(4):
            i, j = divmod(ij, 2)
            nc.sync.dma_start(pv[:, ij, :, :], pv_d[i, j, :, :, :])
        nc.sync.dma_start(wsb[:, :], waff_d[:, :])
        for cb in range(BCH):
            nc.sync.dma_start(
                morT_all[:, cb, :],
                mor_d[ts(cb, 128), :].rearrange("b d -> d b"),
            )
        nc.sync.dma_start(negsel[:, :, :], nsel_d[:, :, :])

        # ---------- phase A: mixture weights ----------
        eabf = small.tile([1, 9], F32)
        sums = small.tile([1, 3], F32)
        nc.scalar.activation(eabf[0:1, 0:2], abf[0:1, 0:2], EXP, accum_out=sums[0:1, 0:1])
        nc.scalar.activation(eabf[0:1, 2:4], abf[0:1, 2:4], EXP, accum_out=sums[0:1, 1:2])
        nc.scalar.activation(eabf[0:1, 4:9], abf[0:1, 4:9], EXP, accum_out=sums[0:1, 2:3])
        rsum = small.tile([1, 3], F32)
        nc.vector.reciprocal(rsum[0:1, :], sums[0:1, :])
        t4 = small.tile([1, 4], F32)
        nc.vector.tensor_mul(
            t4[0:1, :].rearrange("p (i j) -> p i j", i=2),
            eabf[0:1, 0:2].rearrange("p (i j) -> p i j", j=1).to_broadcast((1, 2, 2)),
            eabf[0:1, 2:4].rearrange("p (i j) -> p i j", i=1).to_broadcast((1, 2, 2)),
        )
        t20 = small.tile([1, 20], F32)
        nc.vector.tensor_mul(
            t20[0:1, :].rearrange("p (g k) -> p g k", g=4),
            t4[0:1, :].rearrange("p (g k) -> p g k", k=1).to_broadcast((1, 4, 5)),
            eabf[0:1, 4:9].rearrange("p (g k) -> p g k", g=1).to_broadcast((1, 4, 5)),
        )
        rr = small.tile([1, 1], F32)
        nc.vector.tensor_mul(rr[0:1, :], rsum[0:1, 0:1], rsum[0:1, 1:2])
        rrr = small.tile([1, 1], F32)
        nc.vector.tensor_mul(rrr[0:1, :], rr[0:1, :], rsum[0:1, 2:3])
        w20 = small.tile([1, 20], F32)
        nc.vector.tensor_scalar_mul(w20[0:1, :], t20[0:1, :], rrr[0:1, 0:1])
        nc.gpsimd.partition_broadcast(w_bcast[:, :], w20[0:1, :])

        # ---------- phase B: pivots -> wC -> AllGather (fire ASAP) ----------
        pvE = pvp.tile([128, 20, N], BF16)
        sP = pvp.tile([128, 20], F32)
        for g in range(20):
            nc.scalar.activation(pvE[:, g, :], pv[:, g // 5, g % 5, :], EXP,
                                 accum_out=sP[:, g:g + 1])
        rP = pvp.tile([128, 20], F32)
        nc.vector.reciprocal(rP[:, :], sP[:, :])
        rPw = pvp.tile([128, 20], F32)
        nc.vector.tensor_mul(rPw[:, :], rP[:, :], w_bcast[:, :])
        accA = pvp.tile([128, N], F32)
        accB = pvp.tile([128, N], F32)
        nc.vector.tensor_scalar_mul(accA[:, :], pvE[:, 0, :], rPw[:, 0:1])
        cur, nxt = accA, accB
        for g in range(1, 20):
            nc.vector.scalar_tensor_tensor(
                out=nxt[:, :], in0=pvE[:, g, :], scalar=rPw[:, g:g + 1],
                in1=cur[:, :], op0=ALU.mult, op1=ALU.add,
            )
            cur, nxt = nxt, cur
        wCl = pvp.tile([128, N], BF16)
        nc.vector.tensor_tensor_scan(
            wCl[:, :], data0=cur[:, :], data1=cur[:, :], initial=0.0,
            op0=ALU.add, op1=ALU.bypass,
        )
        dram = ctx.enter_context(tc.tile_pool(name="dram", bufs=1, space="DRAM"))
        wc_in = dram.tile([128, N], BF16)
        wc_out = nc.dram_tensor("wc_gath", [B, N], BF16,
                                addr_space="Shared").ap()
        nc.sync.dma_start(wc_in[:, :], wCl[:, :])
        nc.gpsimd.collective_compute(
            "AllGather", ALU.bypass,
            replica_groups=[list(range(NCORES))],
            ins=[wc_in[:, :].opt()], outs=[wc_out[:, :].opt()],
        )
        nc.sync.dma_start(
            wc_sb[:, :, :],
            wc_out[:, :].rearrange("(c p) n -> p c n", p=128),
        )
        pvp.release()

        # ---------- phase C: attention, normalized in-logits ----------
        psB = tc.alloc_tile_pool(name="psB", bufs=2, space="PSUM")
        psT = tc.alloc_tile_pool(name="psT", bufs=2, space="PSUM")

        # pass A: row sums of exp(logits) for every chunk
        for cb in range(BCH):
            lg_ps = psB.tile([128, V], F32, tag="lg_ps", name=f"lgp{cb}")
            nc.tensor.matmul(lg_ps[:, :], lhsT=morT_all[:, cb, :], rhs=wsb[:, :],
                             start=True, stop=True)
            E = bp.tile([128, V], BF16, tag="E", name=f"E{cb}")
            nc.scalar.activation(E[:, :], lg_ps[:, :], EXP,
                                 accum_out=sE_all[:, cb:cb + 1])
        seT_ps = psT.tile([8, 128], F32, tag="seT", name="seT")
        nc.tensor.transpose(seT_ps[:, :], sE_all[:, :], ident[:, :])
        nc.scalar.activation(lnS_all[:, :], seT_ps[:, :], LN)

        # pass B: logitsT - lnS, exp -> fp8 normalized attnT
        for cb in range(BCH):
            for vc in range(4):
                pT = psT.tile([128, 128], F32, tag="pT", name=f"pT{cb}_{vc}")
                nc.tensor.matmul(pT[:, :], lhsT=wsb[:, ts(vc, 128)],
                                 rhs=morT_all[:, cb, :], start=True, stop=False)
                nc.tensor.matmul(pT[:, :], lhsT=negsel[:, cb, :],
                                 rhs=lnS_all[:, :], start=False, stop=True,
                                 skip_group_check=True)
                nc.scalar.activation(attnT[:, vc, ts(cb, 128)], pT[:, :], EXP)
        psT.release()
        psB.release()

        # ---------- phase D: main loop ----------
        stp = ctx.enter_context(tc.tile_pool(name="stem", bufs=5))
        otp = ctx.enter_context(tc.tile_pool(name="outp", bufs=3))
        prp = ctx.enter_context(tc.tile_pool(name="prod", bufs=3))
        vqp = ctx.enter_context(tc.tile_pool(name="vq", bufs=2))
        psD = ctx.enter_context(tc.tile_pool(name="psD", bufs=2, space="PSUM"))
        dlp = ctx.enter_context(tc.tile_pool(name="delta", bufs=14))
        rwp = ctx.enter_context(tc.tile_pool(name="draw", bufs=2))

        NH = HALF // PSW        # 2 psum tiles per (cb, round)
        gi = 0
        for r in range(2):
            vq = vqp.tile([128, 4, HALF], vocab_dt)
            for vc in range(4):
                nc.sync.dma_start(
                    vq[:, vc, :],
                    vocab_d[ts(vc, 128), ts(r, DHALF), :].rearrange("p d n -> p (d n)"),
                )
            for cb in range(BCH):
                stem_t = stp.tile([128, HALF], BF16)
                nc.sync.dma_start(
                    stem_t[:, :],
                    stem_d[ts(cb, 128), ts(r, DHALF), :].rearrange("p d n -> p (d n)"),
                )
                for h in range(NH):
                    pe_stem = STEM_PE_MOD > 0 and gi % STEM_PE_MOD == 0
                    ps = psD.tile([128, PSW], F32)
                    nt = PSW // 512
                    if USE_DR:
                        for c in range(2):
                            for t in range(nt):
                                col = h * PSW + t * 512
                                nc.tensor.matmul(
                                    ps[:, ts(t, 512)],
                                    lhsT=attnT[:, 2 * c:2 * c + 2, ts(cb, 128)],
                                    rhs=vq[:, 2 * c:2 * c + 2, col:col + 512],
                                    start=(c == 0),
                                    stop=(c == 1 and not pe_stem),
                                    perf_mode=DR,
                                )
                    else:
                        for vc in range(4):
                            for t in range(nt):
                                col = h * PSW + t * 512
                                nc.tensor.matmul(
                                    ps[:, ts(t, 512)],
                                    lhsT=attnT[:, vc:vc + 1, ts(cb, 128)],
                                    rhs=vq[:, vc, col:col + 512],
                                    start=(vc == 0),
                                    stop=(vc == 3 and not pe_stem),
                                )
                    if pe_stem:
                        for t in range(nt):
                            col = h * PSW + t * 512
                            nc.tensor.matmul(
                                ps[:, ts(t, 512)],
                                lhsT=negI[:, :],
                                rhs=stem_t[:, col:col + 512],
                                start=False, stop=True,
                                skip_group_check=True,
                            )
                    delta_t = dlp.tile([128, PSW], BF16)
                    if pe_stem:
                        nc.scalar.copy(delta_t[:, :], ps[:, :])
                    else:
                        raw_t = rwp.tile([128, PSW], BF16)
                        nc.scalar.copy(raw_t[:, :], ps[:, :])
                        nc.vector.tensor_sub(delta_t[:, :], raw_t[:, :],
                                             stem_t[:, ts(h, PSW)])
                    gi += 1
                    prod = prp.tile([128, PSW], BF16)
                    nc.vector.tensor_mul(
                        prod[:, :].rearrange("p (a n) -> p a n", n=N),
                        delta_t[:, :].rearrange("p (a n) -> p a n", n=N),
                        wc_sb[:, cb:cb + 1, :].to_broadcast((128, PSW // N, N)),
                    )
                    out_t = otp.tile([128, PSW], BF16)
                    nc.vector.tensor_add(out_t[:, :], prod[:, :],
                                         stem_t[:, ts(h, PSW)])
                    nc.sync.dma_start(
                        out_d[ts(cb, 128), bass.ds(r * DHALF + h * (PSW // N), PSW // N), :]
                        .rearrange("p d n -> p (d n)"),
                        out_t[:, :],
                    )

    nc.compile()
    _CACHE[key] = nc
    return nc


def kernel(stem_form, morphosyn, pivot_logits, W_affix, affix_vocab,
           alpha, beta, phi, max_len):
    global LAST_RESULT
    stem_form = np.ascontiguousarray(np.asarray(stem_form, dtype=np.float32))
    morphosyn = np.ascontiguousarray(np.asarray(morphosyn, dtype=np.float32))
    pivot_logits = np.ascontiguousarray(np.asarray(pivot_logits, dtype=np.float32))
    W_affix = np.ascontiguousarray(np.asarray(W_affix, dtype=np.float32))
    affix_vocab = np.ascontiguousarray(np.asarray(affix_vocab, dtype=np.float32))
    abf = np.concatenate([
        np.asarray(alpha, np.float32).ravel(),
        np.asarray(beta, np.float32).ravel(),
        np.asarray(phi, np.float32).ravel(),
    ]).reshape(1, 9)

    nc = _build()

    nsel = np.zeros((8, 8, 128), dtype=np.float32)
    for cb in range(8):
        nsel[cb, cb, :] = -1.0
    stem_np = stem_form.astype(ml_dtypes.bfloat16)
    vocab_np = affix_vocab.astype(
        ml_dtypes.float8_e4m3 if USE_DR else ml_dtypes.bfloat16)
    pivot_np = pivot_logits.astype(ml_dtypes.bfloat16) if PIVOT_BF16 else pivot_logits

    in_maps = []
    for c in range(NCORES):
        dlo, dhi = c * DLOC, (c + 1) * DLOC
        in_maps.append({
            "stem": np.ascontiguousarray(stem_np[:, dlo:dhi, :]),
            "vocab": np.ascontiguousarray(vocab_np[:, dlo:dhi, :]),
            "morpho": morphosyn,
            "waffix": W_affix,
            "pivot": np.ascontiguousarray(pivot_np[:, :, c * 128:(c + 1) * 128, :, :]),
            "abf": abf,
            "negsel": nsel,
        })

    LAST_RESULT = run_bass_kernel_spmd(nc, in_maps, core_ids=list(range(NCORES)))
    outs = [LAST_RESULT.results[c]["out"] for c in range(NCORES)]
    out = np.concatenate([o.astype(np.float32) for o in outs], axis=1)
    return np.ascontiguousarray(out)


# revision 16
# speedup vs baseline: 1.5571x; 1.5571x over previous
"""Trainium2 Bass kernel for nn_MixtureCogrammar.

Computation (reference):
    attn  = softmax(morphosyn @ W_affix)                    [B, V]
    affix = attn @ affix_vocab.reshape(V, D*N)              [B, D, N]
    wC    = cumsum_n( sum_{ijk} a_i b_j f_k softmax(pivot_logits[i,j,:,k,:]) )
    out   = stem + wC * (affix - stem)

Distribution: D is sharded over the 8 cores (D_local = 32). Every core
computes the full attention (cheap); the pivot/wC path is batch-sharded
with an AllGather; affix_vocab / stem / out are D-sharded.

Per-core structure (v2):
  - pivot softmax -> wC fires the AllGather as early as possible (it is
    the only cross-core dependency and pays the launch-skew barrier)
  - attention: logits come out of the PE already log-softmax-normalized
    (a K=8 row-selector matmul subtracts ln(sum(exp)) inside the psum
    accumulation), so exp() on ScalarE writes *normalized* fp8 attnT
    directly -- no transposes of attn, no per-partition rescale later
  - the big matmul runs fp8 DoubleRow (contraction 256/MM, 2 MMs per
    512-col psum group instead of 4 bf16 MMs), and a third bf16 matmul
    with -I weights subtracts stem inside the accumulation, so PSUM
    holds delta = affix - stem directly
  - ScalarE evacuates delta (psum->bf16 copy); DVE only does
    prod = delta*wC and out = prod + stem; output streams out per
    2048-col tile
"""

import os
import sys

import numpy as np

for _p in ("/opt/trn_rl_repo",):
    if os.path.isdir(_p) and _p not in sys.path:
        sys.path.append(_p)

import concourse.bass as bass  # noqa: E402
import concourse.tile as tile  # noqa: E402
from concourse import bacc, mybir  # noqa: E402
from concourse.bass import ts  # noqa: E402
from concourse.bass_utils import run_bass_kernel_spmd  # noqa: E402
from concourse.masks import make_identity  # noqa: E402

import ml_dtypes  # noqa: E402

B, D, N, DM, V = 1024, 256, 256, 128, 512
NCORES = 8
DLOC = D // NCORES          # 32 d-values per core
BCH = B // 128              # 8 batch chunks
DN = DLOC * N               # 8192 free elems per core
HALF = DN // 2              # 4096 per round
DHALF = DLOC // 2           # 16 d-values per round
PSW = 2048                  # one psum tile = 4 banks

F32 = mybir.dt.float32
F32R = mybir.dt.float32r
BF16 = mybir.dt.bfloat16
FP8 = mybir.dt.float8e4
EXP = mybir.ActivationFunctionType.Exp
LN = mybir.ActivationFunctionType.Ln
COPY = mybir.ActivationFunctionType.Copy
ALU = mybir.AluOpType
DR = mybir.MatmulPerfMode.DoubleRow

# knobs
USE_DR = True        # fp8 DoubleRow for the big matmul (else bf16)
STEM_PE_MOD = 1      # 1: every tile subtracts stem via -I matmul;
                     # k>1: only tiles with gi%k==0; 0: never (DVE sub)
PIVOT_BF16 = True    # host-cast pivot logits to bf16

LAST_RESULT = None   # BassKernelResults of the last run (exec_time_ns etc.)

_CACHE = {}


def _build():
    key = (USE_DR, STEM_PE_MOD, PIVOT_BF16)
    if key in _CACHE:
        return _CACHE[key]

    vocab_dt = FP8 if USE_DR else BF16
    pivot_dt = BF16 if PIVOT_BF16 else F32

    nc = bacc.Bacc("TRN2", target_bir_lowering=False, debug=False,
                   num_devices=NCORES)

    stem_d = nc.dram_tensor("stem", [B, DLOC, N], BF16, kind="ExternalInput").ap()
    vocab_d = nc.dram_tensor("vocab", [V, DLOC, N], vocab_dt, kind="ExternalInput").ap()
    mor_d = nc.dram_tensor("morpho", [DM, B], F32R, kind="ExternalInput").ap()
    waff_d = nc.dram_tensor("waffix", [DM, V], F32R, kind="ExternalInput").ap()
    pv_d = nc.dram_tensor("pivot", [2, 2, 128, 5, N], pivot_dt, kind="ExternalInput").ap()
    abf_d = nc.dram_tensor("abf", [1, 9], F32, kind="ExternalInput").ap()
    nsel_d = nc.dram_tensor("negsel", [8, 8, 128], F32R, kind="ExternalInput").ap()
    out_d = nc.dram_tensor("out", [B, DLOC, N], BF16, kind="ExternalOutput").ap()

    from contextlib import ExitStack

    with tile.TileContext(nc) as tc, ExitStack() as ctx:
        const = ctx.enter_context(tc.tile_pool(name="const", bufs=1))

        ident = const.tile([128, 128], F32)
        make_identity(nc, ident[:, :])
        # -64*I: the fp8 attnT carries a x64 scale (keeps softmax weights out
        # of fp8-subnormal range); the stem matmul and the evac compensate.
        negI = const.tile([128, 128], BF16)
        nc.vector.tensor_scalar_mul(negI[:, :], ident[:, :], -64.0)
        ln64 = const.tile([128, 1], F32)
        nc.vector.memset(ln64[:, :], 4.15888308335967)

        attnT = const.tile([128, 4, B], FP8)       # [v_part, vc, b] normalized
        wc_sb = const.tile([128, BCH, N], BF16)    # [b_part, cb, n]
        w_bcast = const.tile([128, 20], F32)
        wsb = const.tile([128, V], F32R)           # W_affix resident
        morT_all = const.tile([128, BCH, 128], F32R)  # [dm, cb, b] via DMA transpose
        sE_all = const.tile([128, BCH], F32)       # sum(exp(logits)) per b
        lnS_all = const.tile([8, 128], F32R)       # ln of the above, [cb, b]
        negsel = const.tile([8, 8, 128], F32R)     # -row-selector weights

        # ---------- input DMAs ----------
        small = ctx.enter_context(tc.tile_pool(name="small", bufs=1))
        bp = ctx.enter_context(tc.tile_pool(name="attn", bufs=2))
        pvp = tc.alloc_tile_pool(name="pv", bufs=1)
        pv = pvp.tile([128, 4, 5, N], pivot_dt)
        abf = small.tile([1, 9], F32)
        nc.sync.dma_start(abf[0:1, :], abf_d[:, :])
        for ij in range(4):
            i, j = divmod(ij, 2)
            nc.sync.dma_start(pv[:, ij, :, :], pv_d[i, j, :, :, :])
        nc.sync.dma_start(wsb[:, :], waff_d[:, :])
        nc.sync.dma_start(
            morT_all[:, :, :],
            mor_d[:, :].rearrange("d (c b) -> d c b", b=128),
        )
        nc.sync.dma_start(negsel[:, :, :], nsel_d[:, :, :])

        # ---------- phase A: mixture weights ----------
        eabf = small.tile([1, 9], F32)
        sums = small.tile([1, 3], F32)
        nc.scalar.activation(eabf[0:1, 0:2], abf[0:1, 0:2], EXP, accum_out=sums[0:1, 0:1])
        nc.scalar.activation(eabf[0:1, 2:4], abf[0:1, 2:4], EXP, accum_out=sums[0:1, 1:2])
        nc.scalar.activation(eabf[0:1, 4:9], abf[0:1, 4:9], EXP, accum_out=sums[0:1, 2:3])
        rsum = small.tile([1, 3], F32)
        nc.vector.reciprocal(rsum[0:1, :], sums[0:1, :])
        t4 = small.tile([1, 4], F32)
        nc.vector.tensor_mul(
            t4[0:1, :].rearrange("p (i j) -> p i j", i=2),
            eabf[0:1, 0:2].rearrange("p (i j) -> p i j", j=1).to_broadcast((1, 2, 2)),
            eabf[0:1, 2:4].rearrange("p (i j) -> p i j", i=1).to_broadcast((1, 2, 2)),
        )
        t20 = small.tile([1, 20], F32)
        nc.vector.tensor_mul(
            t20[0:1, :].rearrange("p (g k) -> p g k", g=4),
            t4[0:1, :].rearrange("p (g k) -> p g k", k=1).to_broadcast((1, 4, 5)),
            eabf[0:1, 4:9].rearrange("p (g k) -> p g k", g=1).to_broadcast((1, 4, 5)),
        )
        rr = small.tile([1, 1], F32)
        nc.vector.tensor_mul(rr[0:1, :], rsum[0:1, 0:1], rsum[0:1, 1:2])
        rrr = small.tile([1, 1], F32)
        nc.vector.tensor_mul(rrr[0:1, :], rr[0:1, :], rsum[0:1, 2:3])
        w20 = small.tile([1, 20], F32)
        nc.vector.tensor_scalar_mul(w20[0:1, :], t20[0:1, :], rrr[0:1, 0:1])
        nc.gpsimd.partition_broadcast(w_bcast[:, :], w20[0:1, :])

        # ---------- phase B: pivots -> wC -> AllGather (fire ASAP) ----------
        pvE = pvp.tile([128, 20, N], BF16)
        sP = pvp.tile([128, 20], F32)
        for g in range(20):
            nc.scalar.activation(pvE[:, g, :], pv[:, g // 5, g % 5, :], EXP,
                                 accum_out=sP[:, g:g + 1])
        rP = pvp.tile([128, 20], F32)
        nc.vector.reciprocal(rP[:, :], sP[:, :])
        rPw = pvp.tile([128, 20], F32)
        nc.vector.tensor_mul(rPw[:, :], rP[:, :], w_bcast[:, :])
        accA = pvp.tile([128, N], F32)
        accB = pvp.tile([128, N], F32)
        nc.vector.tensor_scalar_mul(accA[:, :], pvE[:, 0, :], rPw[:, 0:1])
        cur, nxt = accA, accB
        for g in range(1, 20):
            nc.vector.scalar_tensor_tensor(
                out=nxt[:, :], in0=pvE[:, g, :], scalar=rPw[:, g:g + 1],
                in1=cur[:, :], op0=ALU.mult, op1=ALU.add,
            )
            cur, nxt = nxt, cur
        wCl = pvp.tile([128, N], BF16)
        nc.vector.tensor_tensor_scan(
            wCl[:, :], data0=cur[:, :], data1=cur[:, :], initial=0.0,
            op0=ALU.add, op1=ALU.bypass,
        )
        dram = ctx.enter_context(tc.tile_pool(name="dram", bufs=1, space="DRAM"))
        wc_in = dram.tile([128, N], BF16)
        wc_out = nc.dram_tensor("wc_gath", [B, N], BF16,
                                addr_space="Shared").ap()
        nc.sync.dma_start(wc_in[:, :], wCl[:, :])
        nc.gpsimd.collective_compute(
            "AllGather", ALU.bypass,
            replica_groups=[list(range(NCORES))],
            ins=[wc_in[:, :].opt()], outs=[wc_out[:, :].opt()],
        )
        nc.sync.dma_start(
            wc_sb[:, :, :],
            wc_out[:, :].rearrange("(c p) n -> p c n", p=128),
        )
        pvp.release()

        # ---------- phase C: attention, normalized in-logits ----------
        psB = tc.alloc_tile_pool(name="psB", bufs=2, space="PSUM")
        psT = tc.alloc_tile_pool(name="psT", bufs=2, space="PSUM")

        # pass A: row sums of exp(logits) for every chunk
        for cb in range(BCH):
            lg_ps = psB.tile([128, V], F32, tag="lg_ps", name=f"lgp{cb}")
            nc.tensor.matmul(lg_ps[:, :], lhsT=morT_all[:, cb, :], rhs=wsb[:, :],
                             start=True, stop=True)
            E = bp.tile([128, V], BF16, tag="E", name=f"E{cb}")
            nc.scalar.activation(E[:, :], lg_ps[:, :], EXP,
                                 accum_out=sE_all[:, cb:cb + 1])
        seT_ps = psT.tile([8, 128], F32, tag="seT", name="seT")
        nc.tensor.transpose(seT_ps[:, :], sE_all[:, :], ident[:, :])
        nc.scalar.activation(lnS_all[:, :], seT_ps[:, :], LN)

        # pass B: logitsT - lnS, exp -> fp8 normalized attnT
        for cb in range(BCH):
            for vc in range(4):
                pT = psT.tile([128, 128], F32, tag="pT", name=f"pT{cb}_{vc}")
                nc.tensor.matmul(pT[:, :], lhsT=wsb[:, ts(vc, 128)],
                                 rhs=morT_all[:, cb, :], start=True, stop=False)
                nc.tensor.matmul(pT[:, :], lhsT=negsel[:, cb, :],
                                 rhs=lnS_all[:, :], start=False, stop=True,
                                 skip_group_check=True)
                # attnT = 64 * softmax (x64 keeps fp8 out of subnormals)
                nc.scalar.activation(attnT[:, vc, ts(cb, 128)], pT[:, :], EXP,
                                     bias=ln64[:, 0:1])
        psT.release()
        psB.release()

        # ---------- phase D: main loop ----------
        stp = ctx.enter_context(tc.tile_pool(name="stem", bufs=5))
        otp = ctx.enter_context(tc.tile_pool(name="outp", bufs=3))
        prp = ctx.enter_context(tc.tile_pool(name="prod", bufs=3))
        vqp = ctx.enter_context(tc.tile_pool(name="vq", bufs=2))
        psD = ctx.enter_context(tc.tile_pool(name="psD", bufs=2, space="PSUM"))
        dlp = ctx.enter_context(tc.tile_pool(name="delta", bufs=14))
        rwp = ctx.enter_context(tc.tile_pool(name="draw", bufs=2))

        NH = HALF // PSW        # 2 psum tiles per (cb, round)
        gi = 0
        for r in range(2):
            vq = vqp.tile([128, 4, HALF], vocab_dt)
            for vc in range(4):
                nc.sync.dma_start(
                    vq[:, vc, :],
                    vocab_d[ts(vc, 128), ts(r, DHALF), :].rearrange("p d n -> p (d n)"),
                )
            for cb in range(BCH):
                stem_t = stp.tile([128, HALF], BF16)
                nc.sync.dma_start(
                    stem_t[:, :],
                    stem_d[ts(cb, 128), ts(r, DHALF), :].rearrange("p d n -> p (d n)"),
                )
                for h in range(NH):
                    pe_stem = STEM_PE_MOD > 0 and gi % STEM_PE_MOD == 0
                    ps = psD.tile([128, PSW], F32)
                    nt = PSW // 512
                    # weight-major order: each lhsT is loaded once per tile
                    # (DoubleRow consumes both weight regs, so per-MM weight
                    # swaps cannot be hidden behind the matmuls)
                    if USE_DR:
                        for c in range(2):
                            for t in range(nt):
                                col = h * PSW + t * 512
                                nc.tensor.matmul(
                                    ps[:, ts(t, 512)],
                                    lhsT=attnT[:, 2 * c:2 * c + 2, ts(cb, 128)],
                                    rhs=vq[:, 2 * c:2 * c + 2, col:col + 512],
                                    start=(c == 0),
                                    stop=(c == 1 and not pe_stem),
                                    perf_mode=DR,
                                )
                    else:
                        for vc in range(4):
                            for t in range(nt):
                                col = h * PSW + t * 512
                                nc.tensor.matmul(
                                    ps[:, ts(t, 512)],
                                    lhsT=attnT[:, vc:vc + 1, ts(cb, 128)],
                                    rhs=vq[:, vc, col:col + 512],
                                    start=(vc == 0),
                                    stop=(vc == 3 and not pe_stem),
                                )
                    if pe_stem:
                        for t in range(nt):
                            col = h * PSW + t * 512
                            nc.tensor.matmul(
                                ps[:, ts(t, 512)],
                                lhsT=negI[:, :],
                                rhs=stem_t[:, col:col + 512],
                                start=False, stop=True,
                                skip_group_check=True,
                            )
                    delta_t = dlp.tile([128, PSW], BF16)
                    if pe_stem:
                        nc.scalar.mul(delta_t[:, :], ps[:, :], 1.0 / 64.0)
                    else:
                        raw_t = rwp.tile([128, PSW], BF16)
                        nc.scalar.mul(raw_t[:, :], ps[:, :], 1.0 / 64.0)
                        nc.vector.tensor_sub(delta_t[:, :], raw_t[:, :],
                                             stem_t[:, ts(h, PSW)])
                    gi += 1
                    prod = prp.tile([128, PSW], BF16)
                    nc.vector.tensor_mul(
                        prod[:, :].rearrange("p (a n) -> p a n", n=N),
                        delta_t[:, :].rearrange("p (a n) -> p a n", n=N),
                        wc_sb[:, cb:cb + 1, :].to_broadcast((128, PSW // N, N)),
                    )
                    out_t = otp.tile([128, PSW], BF16)
                    nc.vector.tensor_add(out_t[:, :], prod[:, :],
                                         stem_t[:, ts(h, PSW)])
                    nc.sync.dma_start(
                        out_d[ts(cb, 128), bass.ds(r * DHALF + h * (PSW // N), PSW // N), :]
                        .rearrange("p d n -> p (d n)"),
                        out_t[:, :],
                    )

    nc.compile()
    _CACHE[key] = nc
    return nc


def kernel(stem_form, morphosyn, pivot_logits, W_affix, affix_vocab,
           alpha, beta, phi, max_len):
    global LAST_RESULT
    stem_form = np.ascontiguousarray(np.asarray(stem_form, dtype=np.float32))
    morphosyn = np.ascontiguousarray(np.asarray(morphosyn, dtype=np.float32))
    pivot_logits = np.ascontiguousarray(np.asarray(pivot_logits, dtype=np.float32))
    W_affix = np.ascontiguousarray(np.asarray(W_affix, dtype=np.float32))
    affix_vocab = np.ascontiguousarray(np.asarray(affix_vocab, dtype=np.float32))
    abf = np.concatenate([
        np.asarray(alpha, np.float32).ravel(),
        np.asarray(beta, np.float32).ravel(),
        np.asarray(phi, np.float32).ravel(),
    ]).reshape(1, 9)
    morT = np.ascontiguousarray(morphosyn.T)

    nc = _build()

    nsel = np.zeros((8, 8, 128), dtype=np.float32)
    for cb in range(8):
        nsel[cb, cb, :] = -1.0
    stem_np = stem_form.astype(ml_dtypes.bfloat16)
    vocab_np = affix_vocab.astype(
        ml_dtypes.float8_e4m3 if USE_DR else ml_dtypes.bfloat16)
    pivot_np = pivot_logits.astype(ml_dtypes.bfloat16) if PIVOT_BF16 else pivot_logits

    in_maps = []
    for c in range(NCORES):
        dlo, dhi = c * DLOC, (c + 1) * DLOC
        in_maps.append({
            "stem": np.ascontiguousarray(stem_np[:, dlo:dhi, :]),
            "vocab": np.ascontiguousarray(vocab_np[:, dlo:dhi, :]),
            "morpho": morT,
            "waffix": W_affix,
            "pivot": np.ascontiguousarray(pivot_np[:, :, c * 128:(c + 1) * 128, :, :]),
            "abf": abf,
            "negsel": nsel,
        })

    LAST_RESULT = run_bass_kernel_spmd(nc, in_maps, core_ids=list(range(NCORES)))
    outs = [LAST_RESULT.results[c]["out"] for c in range(NCORES)]
    out = np.concatenate([o.astype(np.float32) for o in outs], axis=1)
    return np.ascontiguousarray(out)


# revision 18
# speedup vs baseline: 1.6858x; 1.0827x over previous
"""Trainium2 Bass kernel for nn_MixtureCogrammar.

Computation (reference):
    attn  = softmax(morphosyn @ W_affix)                    [B, V]
    affix = attn @ affix_vocab.reshape(V, D*N)              [B, D, N]
    wC    = cumsum_n( sum_{ijk} a_i b_j f_k softmax(pivot_logits[i,j,:,k,:]) )
    out   = stem + wC * (affix - stem)

Distribution: D is sharded over the 8 cores (D_local = 32). Every core
computes the full attention (cheap); the pivot/wC path is batch-sharded
with an AllGather; affix_vocab / stem / out are D-sharded.

Per-core structure (v2):
  - pivot softmax -> wC fires the AllGather as early as possible (it is
    the only cross-core dependency and pays the launch-skew barrier)
  - attention: logits come out of the PE already log-softmax-normalized
    (a K=8 row-selector matmul subtracts ln(sum(exp)) inside the psum
    accumulation), so exp() on ScalarE writes *normalized* fp8 attnT
    directly -- no transposes of attn, no per-partition rescale later
  - the big matmul runs fp8 DoubleRow (contraction 256/MM, 2 MMs per
    512-col psum group instead of 4 bf16 MMs), and a third bf16 matmul
    with -I weights subtracts stem inside the accumulation, so PSUM
    holds delta = affix - stem directly
  - ScalarE evacuates delta (psum->bf16 copy); DVE only does
    prod = delta*wC and out = prod + stem; output streams out per
    2048-col tile
"""

import os
import sys

import numpy as np

for _p in ("/opt/trn_rl_repo",):
    if os.path.isdir(_p) and _p not in sys.path:
        sys.path.append(_p)

import concourse.bass as bass  # noqa: E402
import concourse.tile as tile  # noqa: E402
from concourse import bacc, mybir  # noqa: E402
from concourse.bass import ts  # noqa: E402
from concourse.bass_utils import run_bass_kernel_spmd  # noqa: E402
from concourse.masks import make_identity  # noqa: E402

import ml_dtypes  # noqa: E402

B, D, N, DM, V = 1024, 256, 256, 128, 512
NCORES = 8
DLOC = D // NCORES          # 32 d-values per core
BCH = B // 128              # 8 batch chunks
DN = DLOC * N               # 8192 free elems per core
HALF = DN // 2              # 4096 per round
DHALF = DLOC // 2           # 16 d-values per round
PSW = 2048                  # one psum tile = 4 banks

F32 = mybir.dt.float32
F32R = mybir.dt.float32r
BF16 = mybir.dt.bfloat16
FP8 = mybir.dt.float8e4
EXP = mybir.ActivationFunctionType.Exp
LN = mybir.ActivationFunctionType.Ln
COPY = mybir.ActivationFunctionType.Copy
ALU = mybir.AluOpType
DR = mybir.MatmulPerfMode.DoubleRow

# knobs
USE_DR = True        # fp8 DoubleRow for the big matmul (else bf16)
STEM_PE_MOD = 2      # 1: every tile subtracts stem via -I matmul;
                     # k>1: only tiles with gi%k==0; 0: never (DVE sub)
PIVOT_BF16 = True    # host-cast pivot logits to bf16

LAST_RESULT = None   # BassKernelResults of the last run (exec_time_ns etc.)

_CACHE = {}


def _build():
    key = (USE_DR, STEM_PE_MOD, PIVOT_BF16)
    if key in _CACHE:
        return _CACHE[key]

    vocab_dt = FP8 if USE_DR else BF16
    pivot_dt = BF16 if PIVOT_BF16 else F32

    nc = bacc.Bacc("TRN2", target_bir_lowering=False, debug=False,
                   num_devices=NCORES)

    stem_d = nc.dram_tensor("stem", [B, DLOC, N], BF16, kind="ExternalInput").ap()
    vocab_d = nc.dram_tensor("vocab", [V, DLOC, N], vocab_dt, kind="ExternalInput").ap()
    mor_d = nc.dram_tensor("morpho", [DM, B], F32R, kind="ExternalInput").ap()
    waff_d = nc.dram_tensor("waffix", [DM, V], F32R, kind="ExternalInput").ap()
    pv_d = nc.dram_tensor("pivot", [2, 2, 128, 5, N], pivot_dt, kind="ExternalInput").ap()
    abf_d = nc.dram_tensor("abf", [1, 9], F32, kind="ExternalInput").ap()
    nsel_d = nc.dram_tensor("negsel", [8, 8, 128], F32R, kind="ExternalInput").ap()
    out_d = nc.dram_tensor("out", [B, DLOC, N], BF16, kind="ExternalOutput").ap()

    from contextlib import ExitStack

    with tile.TileContext(nc) as tc, ExitStack() as ctx:
        const = ctx.enter_context(tc.tile_pool(name="const", bufs=1))

        ident = const.tile([128, 128], F32)
        make_identity(nc, ident[:, :])
        # -64*I: the fp8 attnT carries a x64 scale (keeps softmax weights out
        # of fp8-subnormal range); the stem matmul and the evac compensate.
        negI = const.tile([128, 128], BF16)
        nc.vector.tensor_scalar_mul(negI[:, :], ident[:, :], -64.0)
        ln64 = const.tile([128, 1], F32)
        nc.vector.memset(ln64[:, :], 4.15888308335967)

        attnT = const.tile([128, 4, B], FP8)       # [v_part, vc, b] normalized
        wc_sb = const.tile([128, BCH, N], BF16)    # [b_part, cb, n]
        w_bcast = const.tile([128, 20], F32)
        wsb = const.tile([128, V], F32R)           # W_affix resident
        morT_all = const.tile([128, BCH, 128], F32R)  # [dm, cb, b] via DMA transpose
        sE_all = const.tile([128, BCH], F32)       # sum(exp(logits)) per b
        lnS_all = const.tile([8, 128], F32R)       # ln of the above, [cb, b]
        negsel = const.tile([8, 8, 128], F32R)     # -row-selector weights

        # ---------- input DMAs ----------
        small = ctx.enter_context(tc.tile_pool(name="small", bufs=1))
        bp = ctx.enter_context(tc.tile_pool(name="attn", bufs=2))
        pvp = tc.alloc_tile_pool(name="pv", bufs=1)
        pv = pvp.tile([128, 4, 5, N], pivot_dt)
        abf = small.tile([1, 9], F32)
        nc.sync.dma_start(abf[0:1, :], abf_d[:, :])
        for ij in range(4):
            i, j = divmod(ij, 2)
            nc.sync.dma_start(pv[:, ij, :, :], pv_d[i, j, :, :, :])
        nc.sync.dma_start(wsb[:, :], waff_d[:, :])
        nc.sync.dma_start(
            morT_all[:, :, :],
            mor_d[:, :].rearrange("d (c b) -> d c b", b=128),
        )
        nc.sync.dma_start(negsel[:, :, :], nsel_d[:, :, :])

        # ---------- phase A: mixture weights ----------
        eabf = small.tile([1, 9], F32)
        sums = small.tile([1, 3], F32)
        nc.scalar.activation(eabf[0:1, 0:2], abf[0:1, 0:2], EXP, accum_out=sums[0:1, 0:1])
        nc.scalar.activation(eabf[0:1, 2:4], abf[0:1, 2:4], EXP, accum_out=sums[0:1, 1:2])
        nc.scalar.activation(eabf[0:1, 4:9], abf[0:1, 4:9], EXP, accum_out=sums[0:1, 2:3])
        rsum = small.tile([1, 3], F32)
        nc.vector.reciprocal(rsum[0:1, :], sums[0:1, :])
        t4 = small.tile([1, 4], F32)
        nc.vector.tensor_mul(
            t4[0:1, :].rearrange("p (i j) -> p i j", i=2),
            eabf[0:1, 0:2].rearrange("p (i j) -> p i j", j=1).to_broadcast((1, 2, 2)),
            eabf[0:1, 2:4].rearrange("p (i j) -> p i j", i=1).to_broadcast((1, 2, 2)),
        )
        t20 = small.tile([1, 20], F32)
        nc.vector.tensor_mul(
            t20[0:1, :].rearrange("p (g k) -> p g k", g=4),
            t4[0:1, :].rearrange("p (g k) -> p g k", k=1).to_broadcast((1, 4, 5)),
            eabf[0:1, 4:9].rearrange("p (g k) -> p g k", g=1).to_broadcast((1, 4, 5)),
        )
        rr = small.tile([1, 1], F32)
        nc.vector.tensor_mul(rr[0:1, :], rsum[0:1, 0:1], rsum[0:1, 1:2])
        rrr = small.tile([1, 1], F32)
        nc.vector.tensor_mul(rrr[0:1, :], rr[0:1, :], rsum[0:1, 2:3])
        w20 = small.tile([1, 20], F32)
        nc.vector.tensor_scalar_mul(w20[0:1, :], t20[0:1, :], rrr[0:1, 0:1])
        nc.gpsimd.partition_broadcast(w_bcast[:, :], w20[0:1, :])

        # ---------- phase B: pivots -> wC -> AllGather (fire ASAP) ----------
        pvE = pvp.tile([128, 20, N], BF16)
        sP = pvp.tile([128, 20], F32)
        for g in range(20):
            nc.scalar.activation(pvE[:, g, :], pv[:, g // 5, g % 5, :], EXP,
                                 accum_out=sP[:, g:g + 1])
        rP = pvp.tile([128, 20], F32)
        nc.vector.reciprocal(rP[:, :], sP[:, :])
        rPw = pvp.tile([128, 20], F32)
        nc.vector.tensor_mul(rPw[:, :], rP[:, :], w_bcast[:, :])
        accA = pvp.tile([128, N], F32)
        accB = pvp.tile([128, N], F32)
        nc.vector.tensor_scalar_mul(accA[:, :], pvE[:, 0, :], rPw[:, 0:1])
        cur, nxt = accA, accB
        for g in range(1, 20):
            nc.vector.scalar_tensor_tensor(
                out=nxt[:, :], in0=pvE[:, g, :], scalar=rPw[:, g:g + 1],
                in1=cur[:, :], op0=ALU.mult, op1=ALU.add,
            )
            cur, nxt = nxt, cur
        wCl = pvp.tile([128, N], BF16)
        nc.vector.tensor_tensor_scan(
            wCl[:, :], data0=cur[:, :], data1=cur[:, :], initial=0.0,
            op0=ALU.add, op1=ALU.bypass,
        )
        dram = ctx.enter_context(tc.tile_pool(name="dram", bufs=1, space="DRAM"))
        wc_in = dram.tile([128, N], BF16)
        wc_out = nc.dram_tensor("wc_gath", [B, N], BF16,
                                addr_space="Shared").ap()
        nc.sync.dma_start(wc_in[:, :], wCl[:, :])
        nc.gpsimd.collective_compute(
            "AllGather", ALU.bypass,
            replica_groups=[list(range(NCORES))],
            ins=[wc_in[:, :].opt()], outs=[wc_out[:, :].opt()],
        )
        nc.sync.dma_start(
            wc_sb[:, :, :],
            wc_out[:, :].rearrange("(c p) n -> p c n", p=128),
        )
        pvp.release()

        # ---------- phase C: attention, normalized in-logits ----------
        psB = tc.alloc_tile_pool(name="psB", bufs=2, space="PSUM")
        psT = tc.alloc_tile_pool(name="psT", bufs=2, space="PSUM")

        # pass A: row sums of exp(logits) for every chunk
        for cb in range(BCH):
            lg_ps = psB.tile([128, V], F32, tag="lg_ps", name=f"lgp{cb}")
            nc.tensor.matmul(lg_ps[:, :], lhsT=morT_all[:, cb, :], rhs=wsb[:, :],
                             start=True, stop=True)
            E = bp.tile([128, V], BF16, tag="E", name=f"E{cb}")
            nc.scalar.activation(E[:, :], lg_ps[:, :], EXP,
                                 accum_out=sE_all[:, cb:cb + 1])
        seT_ps = psT.tile([8, 128], F32, tag="seT", name="seT")
        nc.tensor.transpose(seT_ps[:, :], sE_all[:, :], ident[:, :])
        nc.scalar.activation(lnS_all[:, :], seT_ps[:, :], LN)

        # pass B: logitsT - lnS, exp -> fp8 normalized attnT
        for cb in range(BCH):
            for vc in range(4):
                pT = psT.tile([128, 128], F32, tag="pT", name=f"pT{cb}_{vc}")
                nc.tensor.matmul(pT[:, :], lhsT=wsb[:, ts(vc, 128)],
                                 rhs=morT_all[:, cb, :], start=True, stop=False)
                nc.tensor.matmul(pT[:, :], lhsT=negsel[:, cb, :],
                                 rhs=lnS_all[:, :], start=False, stop=True,
                                 skip_group_check=True)
                # attnT = 64 * softmax (x64 keeps fp8 out of subnormals)
                nc.scalar.activation(attnT[:, vc, ts(cb, 128)], pT[:, :], EXP,
                                     bias=ln64[:, 0:1])
        psT.release()
        psB.release()

        # ---------- phase D: main loop ----------
        stp = ctx.enter_context(tc.tile_pool(name="stem", bufs=7))
        otp = ctx.enter_context(tc.tile_pool(name="outp", bufs=3))
        prp = ctx.enter_context(tc.tile_pool(name="prod", bufs=3))
        vqp = ctx.enter_context(tc.tile_pool(name="vq", bufs=2))
        psD = ctx.enter_context(tc.tile_pool(name="psD", bufs=2, space="PSUM"))
        dlp = ctx.enter_context(tc.tile_pool(name="delta", bufs=14))
        rwp = ctx.enter_context(tc.tile_pool(name="draw", bufs=2))

        NH = HALF // PSW        # 2 psum tiles per (cb, round)
        gi = 0
        for r in range(2):
            vq = vqp.tile([128, 4, HALF], vocab_dt)
            for vc in range(4):
                nc.sync.dma_start(
                    vq[:, vc, :],
                    vocab_d[ts(vc, 128), ts(r, DHALF), :].rearrange("p d n -> p (d n)"),
                )
            for cb in range(BCH):
                stem_t = stp.tile([128, HALF], BF16)
                nc.sync.dma_start(
                    stem_t[:, :],
                    stem_d[ts(cb, 128), ts(r, DHALF), :].rearrange("p d n -> p (d n)"),
                )
                for h in range(NH):
                    pe_stem = STEM_PE_MOD > 0 and gi % STEM_PE_MOD == 0
                    ps = psD.tile([128, PSW], F32)
                    nt = PSW // 512
                    # weight-major order: each lhsT is loaded once per tile
                    # (DoubleRow consumes both weight regs, so per-MM weight
                    # swaps cannot be hidden behind the matmuls)
                    if USE_DR:
                        for c in range(2):
                            for t in range(nt):
                                col = h * PSW + t * 512
                                nc.tensor.matmul(
                                    ps[:, ts(t, 512)],
                                    lhsT=attnT[:, 2 * c:2 * c + 2, ts(cb, 128)],
                                    rhs=vq[:, 2 * c:2 * c + 2, col:col + 512],
                                    start=(c == 0),
                                    stop=(c == 1 and not pe_stem),
                                    perf_mode=DR,
                                )
                    else:
                        for vc in range(4):
                            for t in range(nt):
                                col = h * PSW + t * 512
                                nc.tensor.matmul(
                                    ps[:, ts(t, 512)],
                                    lhsT=attnT[:, vc:vc + 1, ts(cb, 128)],
                                    rhs=vq[:, vc, col:col + 512],
                                    start=(vc == 0),
                                    stop=(vc == 3 and not pe_stem),
                                )
                    if pe_stem:
                        for t in range(nt):
                            col = h * PSW + t * 512
                            nc.tensor.matmul(
                                ps[:, ts(t, 512)],
                                lhsT=negI[:, :],
                                rhs=stem_t[:, col:col + 512],
                                start=False, stop=True,
                                skip_group_check=True,
                            )
                    delta_t = dlp.tile([128, PSW], BF16)
                    if pe_stem:
                        nc.scalar.mul(delta_t[:, :], ps[:, :], 1.0 / 64.0)
                    else:
                        raw_t = rwp.tile([128, PSW], BF16)
                        nc.scalar.mul(raw_t[:, :], ps[:, :], 1.0 / 64.0)
                        nc.vector.tensor_sub(delta_t[:, :], raw_t[:, :],
                                             stem_t[:, ts(h, PSW)])
                    gi += 1
                    prod = prp.tile([128, PSW], BF16)
                    nc.vector.tensor_mul(
                        prod[:, :].rearrange("p (a n) -> p a n", n=N),
                        delta_t[:, :].rearrange("p (a n) -> p a n", n=N),
                        wc_sb[:, cb:cb + 1, :].to_broadcast((128, PSW // N, N)),
                    )
                    out_t = otp.tile([128, PSW], BF16)
                    nc.vector.tensor_add(out_t[:, :], prod[:, :],
                                         stem_t[:, ts(h, PSW)])
                    nc.sync.dma_start(
                        out_d[ts(cb, 128), bass.ds(r * DHALF + h * (PSW // N), PSW // N), :]
                        .rearrange("p d n -> p (d n)"),
                        out_t[:, :],
                    )

    nc.compile()
    _CACHE[key] = nc
    return nc


def kernel(stem_form, morphosyn, pivot_logits, W_affix, affix_vocab,
           alpha, beta, phi, max_len):
    global LAST_RESULT
    stem_form = np.ascontiguousarray(np.asarray(stem_form, dtype=np.float32))
    morphosyn = np.ascontiguousarray(np.asarray(morphosyn, dtype=np.float32))
    pivot_logits = np.ascontiguousarray(np.asarray(pivot_logits, dtype=np.float32))
    W_affix = np.ascontiguousarray(np.asarray(W_affix, dtype=np.float32))
    affix_vocab = np.ascontiguousarray(np.asarray(affix_vocab, dtype=np.float32))
    abf = np.concatenate([
        np.asarray(alpha, np.float32).ravel(),
        np.asarray(beta, np.float32).ravel(),
        np.asarray(phi, np.float32).ravel(),
    ]).reshape(1, 9)
    morT = np.ascontiguousarray(morphosyn.T)

    nc = _build()

    nsel = np.zeros((8, 8, 128), dtype=np.float32)
    for cb in range(8):
        nsel[cb, cb, :] = -1.0
    stem_np = stem_form.astype(ml_dtypes.bfloat16)
    vocab_np = affix_vocab.astype(
        ml_dtypes.float8_e4m3 if USE_DR else ml_dtypes.bfloat16)
    pivot_np = pivot_logits.astype(ml_dtypes.bfloat16) if PIVOT_BF16 else pivot_logits

    in_maps = []
    for c in range(NCORES):
        dlo, dhi = c * DLOC, (c + 1) * DLOC
        in_maps.append({
            "stem": np.ascontiguousarray(stem_np[:, dlo:dhi, :]),
            "vocab": np.ascontiguousarray(vocab_np[:, dlo:dhi, :]),
            "morpho": morT,
            "waffix": W_affix,
            "pivot": np.ascontiguousarray(pivot_np[:, :, c * 128:(c + 1) * 128, :, :]),
            "abf": abf,
            "negsel": nsel,
        })

    LAST_RESULT = run_bass_kernel_spmd(nc, in_maps, core_ids=list(range(NCORES)))
    outs = [LAST_RESULT.results[c]["out"] for c in range(NCORES)]
    out = np.concatenate([o.astype(np.float32) for o in outs], axis=1)
    return np.ascontiguousarray(out)
